# revision 5
# baseline (speedup 1.0000x reference)
"""MoE layer (top-2 of 8 experts) for 8 Trainium2 NeuronCores.

Strategy: expert-parallel. Host computes the (tiny) router + top-2 dispatch in
numpy; core e runs expert e's FFN over its dispatched tokens (padded to a fixed
capacity) with fp32r matmuls; host combines the two expert outputs per token.

Per-core device kernel, all matmuls [K=128]x[M=128]x[N=512] fp32r:
  gate^T/up^T [H, Ct] = gwT/uwT.T @ xt   (contraction over D, 8 k-tiles)
  h = silu(gate) * up                    (kept in SBUF, [128, 16, Ct] tiles)
  y [Ct, D] = (h.T @ dwT) * p            (contraction over H, 16 k-tiles,
                                          combine-prob scale fused in eviction)
"""

import numpy as np

import concourse.bass as bass
import concourse.mybir as mybir
import concourse.tile as tile
from concourse import bacc
from concourse.bass_utils import run_bass_kernel_spmd

E = 8
TOP_K = 2
B, S, D, H = 4, 2048, 1024, 2048
T = B * S
C = 2560          # per-expert token capacity (seed-0 max count is 2175)
CT = 512          # token tile
P = 128
F32 = mybir.dt.float32
F32R = mybir.dt.float32r
AF = mybir.ActivationFunctionType


def emit_expert_ffn(tc, xt, gwT, uwT, dwT, pv, y, cap=C):
    """Emit one expert's FFN. xt:[D,cap] f32r, gwT/uwT:[D,H] f32r,
    dwT:[H,D] f32r, pv:[P, cap//P] f32, y:[cap,D] f32 out."""
    nc = tc.nc
    nct = cap // CT
    KD = D // P            # 8  k-tiles for gate/up
    KH = H // P            # 16 k-tiles for down
    NH4 = H // 512         # 4  groups of 4 h-blocks
    # superblocks of up to 2 token tiles sharing one weight pass
    sbs = [[c * CT for c in range(s, min(s + 2, nct))] for s in range(0, nct, 2)]

    with (
        tc.tile_pool(name="xpool", bufs=2) as xpool,
        tc.tile_pool(name="wpool", bufs=24) as wpool,
        tc.tile_pool(name="hpool", bufs=2) as hpool,
        tc.tile_pool(name="dpool", bufs=3) as dpool,
        tc.tile_pool(name="tpool", bufs=3) as tpool,
        tc.tile_pool(name="opool", bufs=3) as opool,
        tc.tile_pool(name="ppool", bufs=1) as ppool,
        tc.tile_pool(name="pspool", bufs=8, space="PSUM") as pspool,
    ):
        p_sb = ppool.tile([P, cap // P], F32)
        nc.sync.dma_start(p_sb[:, :], pv[:, :])

        for cts in sbs:
            # ---- load token tiles (transposed: [d-part, kt, token]) ----
            xts = []
            for c0 in cts:
                xt_sb = xpool.tile([P, KD, CT], F32R, name=f"xt_{c0}", tag="xt")
                for kt in range(KD):
                    nc.sync.dma_start(
                        xt_sb[:, kt, :], xt[kt * P:(kt + 1) * P, c0:c0 + CT]
                    )
                xts.append(xt_sb)
            hs = [
                hpool.tile([P, KH, CT], F32R, name=f"h_{c0}", tag="h") for c0 in cts
            ]

            # ---- stage A: gate/up matmuls + silu*mul -> h ----
            for ht4 in range(NH4):
                gts, uts = [], []
                for kt in range(KD):
                    gt = wpool.tile([P, 512], F32R, name=f"g_{ht4}_{kt}", tag="w")
                    nc.sync.dma_start(
                        gt[:, :], gwT[kt * P:(kt + 1) * P, ht4 * 512:(ht4 + 1) * 512]
                    )
                    ut = wpool.tile([P, 512], F32R, name=f"u_{ht4}_{kt}", tag="w")
                    nc.sync.dma_start(
                        ut[:, :], uwT[kt * P:(kt + 1) * P, ht4 * 512:(ht4 + 1) * 512]
                    )
                    gts.append(gt)
                    uts.append(ut)
                for sub in range(4):
                    ht = ht4 * 4 + sub
                    for ci in range(len(cts)):
                        pg = pspool.tile([P, CT], F32, name=f"pg_{ht}_{ci}", tag="ps")
                        pu = pspool.tile([P, CT], F32, name=f"pu_{ht}_{ci}", tag="ps")
                        for kt in range(KD):
                            nc.tensor.matmul(
                                pg[:, :],
                                gts[kt][:, sub * P:(sub + 1) * P],
                                xts[ci][:, kt, :],
                                start=(kt == 0),
                                stop=(kt == KD - 1),
                            )
                        for kt in range(KD):
                            nc.tensor.matmul(
                                pu[:, :],
                                uts[kt][:, sub * P:(sub + 1) * P],
                                xts[ci][:, kt, :],
                                start=(kt == 0),
                                stop=(kt == KD - 1),
                            )
                        tmp = tpool.tile([P, CT], F32, name=f"t_{ht}_{ci}", tag="t")
                        nc.scalar.activation(tmp[:, :], pg[:, :], AF.Silu)
                        nc.vector.tensor_mul(hs[ci][:, ht, :], tmp[:, :], pu[:, :])

            # ---- stage B: down matmuls + prob scale -> y ----
            for dc in range(2):
                pos = {}
                for ci in range(len(cts)):
                    for m in range(CT // P):
                        pos[(ci, m)] = pspool.tile(
                            [P, 512], F32, name=f"po_{dc}_{ci}_{m}", tag="ps"
                        )
                for kh in range(KH):
                    dt_ = dpool.tile([P, 512], F32R, name=f"d_{dc}_{kh}", tag="dw")
                    nc.sync.dma_start(
                        dt_[:, :], dwT[kh * P:(kh + 1) * P, dc * 512:(dc + 1) * 512]
                    )
                    for ci in range(len(cts)):
                        for m in range(CT // P):
                            nc.tensor.matmul(
                                pos[(ci, m)][:, :],
                                hs[ci][:, kh, m * P:(m + 1) * P],
                                dt_[:, :],
                                start=(kh == 0),
                                stop=(kh == KH - 1),
                            )
                for ci, c0 in enumerate(cts):
                    for m in range(CT // P):
                        ot = opool.tile([P, 512], F32, name=f"o_{dc}_{ci}_{m}", tag="o")
                        j = c0 // P + m
                        nc.scalar.mul(ot[:, :], pos[(ci, m)][:, :], p_sb[:, j:j + 1])
                        nc.sync.dma_start(
                            y[c0 + m * P:c0 + (m + 1) * P, dc * 512:(dc + 1) * 512],
                            ot[:, :],
                        )


def build_nc(cap=C, reps_loop=False, max_reps=256):
    """Build the per-core Bass program. With reps_loop, the whole body runs
    inside a For_i whose trip count is read from an int32 input "reps"."""
    nc = bacc.Bacc(None, target_bir_lowering=False)
    with tile.TileContext(nc) as tc:
        xt = nc.dram_tensor("xt", [D, cap], F32R, kind="ExternalInput")
        gwT = nc.dram_tensor("gwT", [D, H], F32R, kind="ExternalInput")
        uwT = nc.dram_tensor("uwT", [D, H], F32R, kind="ExternalInput")
        dwT = nc.dram_tensor("dwT", [H, D], F32R, kind="ExternalInput")
        pv = nc.dram_tensor("pv", [P, cap // P], F32, kind="ExternalInput")
        y = nc.dram_tensor("y", [cap, D], F32, kind="ExternalOutput")
        if reps_loop:
            reps = nc.dram_tensor("reps", [1, 1], mybir.dt.int32, kind="ExternalInput")
            with tc.tile_pool(name="rpool", bufs=1) as rpool:
                r_sb = rpool.tile([1, 1], mybir.dt.int32)
                nc.sync.dma_start(r_sb[:, :], reps[:, :])
                rv = nc.values_load(
                    r_sb[0:1, 0:1],
                    min_val=0,
                    max_val=max_reps,
                    skip_runtime_bounds_check=True,
                )
            with tc.For_i(0, rv, 1):
                emit_expert_ffn(tc, xt, gwT, uwT, dwT, pv, y, cap)
        else:
            emit_expert_ffn(tc, xt, gwT, uwT, dwT, pv, y, cap)
    nc.compile()
    return nc


def route_and_dispatch(x, router_w, cap=C):
    """Host router + top-2 dispatch. Returns per-expert packed inputs and
    combine metadata."""
    logits = x @ router_w.T                      # [T, E]
    t_ar = np.arange(T)
    i1 = np.argmax(logits, axis=1)
    l1 = logits[t_ar, i1]
    lm = logits.copy()
    lm[t_ar, i1] = -np.inf
    i2 = np.argmax(lm, axis=1)
    l2 = lm[t_ar, i2]
    e2 = np.exp(l2 - l1)
    p1 = 1.0 / (1.0 + e2)
    p2 = e2 / (1.0 + e2)

    ee = np.concatenate([i1, i2])                # [2T] expert of each pair
    tt = np.concatenate([t_ar, t_ar])            # [2T] token of each pair
    pp = np.concatenate([p1, p2]).astype(np.float32)
    counts = np.bincount(ee, minlength=E)
    starts = np.zeros(E, np.int64)
    starts[1:] = np.cumsum(counts)[:-1]
    order = np.argsort(ee, kind="stable")
    pos = np.empty(2 * T, np.int64)
    pos[order] = np.arange(2 * T) - starts[ee[order]]
    return ee, tt, pp, pos, counts, starts, order


def kernel(**inputs):
    x = np.ascontiguousarray(
        np.asarray(inputs["hidden_states"], np.float32).reshape(T, D)
    )
    router_w = np.asarray(inputs["router_w"], np.float32)
    gate_w = np.asarray(inputs["gate_w"], np.float32)
    up_w = np.asarray(inputs["up_w"], np.float32)
    down_w = np.asarray(inputs["down_w"], np.float32)

    ee, tt, pp, pos, counts, starts, order = route_and_dispatch(x, router_w)

    in_maps = []
    for e in range(E):
        n_e = min(int(counts[e]), C)
        sel = order[starts[e]:starts[e] + n_e]   # pairs dispatched to core e
        xp = np.zeros((C, D), np.float32)
        xp[:n_e] = x[tt[sel]]
        pvec = np.zeros(C, np.float32)
        pvec[:n_e] = pp[sel]
        in_maps.append(
            {
                "xt": np.ascontiguousarray(xp.T),
                "gwT": np.ascontiguousarray(gate_w[e].T),
                "uwT": np.ascontiguousarray(up_w[e].T),
                "dwT": np.ascontiguousarray(down_w[e].T),
                "pv": np.ascontiguousarray(pvec.reshape(C // P, P).T),
            }
        )

    nc = build_nc()
    res = run_bass_kernel_spmd(nc, in_maps, core_ids=list(range(E)))
    ys = np.stack([res.results[e]["y"] for e in range(E)]).reshape(E * C, D)

    ok = pos < C
    contrib = np.zeros((2 * T, D), np.float32)
    g = ee * C + pos
    contrib[ok] = ys[g[ok]]
    # capacity-overflow fallback (never hit for the seed-0 data): exact fp32
    # host compute for pairs beyond capacity
    if not ok.all():
        for j in np.nonzero(~ok)[0]:
            e = int(ee[j])
            xv = x[tt[j]]
            gate = gate_w[e] @ xv
            up = up_w[e] @ xv
            hv = (gate / (1.0 + np.exp(-gate))) * up
            contrib[j] = (down_w[e] @ hv) * pp[j]
    out = contrib[:T] + contrib[T:]
    return out.reshape(B, S, D).astype(np.float32)


# revision 8
# speedup vs baseline: 1.0244x; 1.0244x over previous
"""MoE layer (top-2 of 8 experts) for 8 Trainium2 NeuronCores.

Strategy: expert-parallel. Host computes the (tiny) router + top-2 dispatch in
numpy; core e runs expert e's FFN over its dispatched tokens (padded to a fixed
capacity) with fp32r matmuls; host combines the two expert outputs per token.

Per-core device kernel, all matmuls [K=128]x[M=128]x[N=512] fp32r:
  gate^T/up^T [H, Ct] = gwT/uwT.T @ xt   (contraction over D, 8 k-tiles)
  h = silu(gate) * up                    (kept in SBUF, [128, 16, Ct] tiles)
  y [Ct, D] = (h.T @ dwT) * p            (contraction over H, 16 k-tiles,
                                          combine-prob scale fused in eviction)
"""

import numpy as np

import concourse.bass as bass
import concourse.mybir as mybir
import concourse.tile as tile
from concourse import bacc
from concourse.bass_utils import run_bass_kernel_spmd

E = 8
TOP_K = 2
B, S, D, H = 4, 2048, 1024, 2048
T = B * S
C = 2560          # per-expert token capacity (seed-0 max count is 2175)
CT = 512          # token tile
P = 128
F32 = mybir.dt.float32
F32R = mybir.dt.float32r
AF = mybir.ActivationFunctionType


def emit_expert_ffn(tc, xt, gwT, uwT, dwT, pv, y, cap=C):
    """Emit one expert's FFN. xt:[D,cap] f32r, gwT/uwT:[D,H] f32r,
    dwT:[H,D] f32r, pv:[P, cap//P] f32, y:[cap,D] f32 out."""
    nc = tc.nc
    nct = cap // CT
    KD = D // P            # 8  k-tiles for gate/up
    KH = H // P            # 16 k-tiles for down
    NH4 = H // 512         # 4  groups of 4 h-blocks
    # superblocks of up to 2 token tiles sharing one weight pass
    sbs = [[c * CT for c in range(s, min(s + 2, nct))] for s in range(0, nct, 2)]

    with (
        tc.tile_pool(name="xpool", bufs=24) as xpool,
        tc.tile_pool(name="wpool", bufs=20) as wpool,
        tc.tile_pool(name="hpool", bufs=40) as hpool,
        tc.tile_pool(name="dpool", bufs=4) as dpool,
        tc.tile_pool(name="tpool", bufs=3) as tpool,
        tc.tile_pool(name="opool", bufs=3) as opool,
        tc.tile_pool(name="ppool", bufs=1) as ppool,
        tc.tile_pool(name="pspool", bufs=8, space="PSUM") as pspool,
    ):
        p_sb = ppool.tile([P, cap // P], F32)
        nc.sync.dma_start(p_sb[:, :], pv[:, :])

        for cts in sbs:
            # ---- load token tiles (transposed: [d-part, token]) ----
            xts = []
            for c0 in cts:
                kts = []
                for kt in range(KD):
                    x_t = xpool.tile([P, CT], F32R, name=f"xt_{c0}_{kt}", tag="xt")
                    nc.sync.dma_start(
                        x_t[:, :], xt[kt * P:(kt + 1) * P, c0:c0 + CT]
                    )
                    kts.append(x_t)
                xts.append(kts)
            hs = [
                [
                    hpool.tile([P, CT], F32R, name=f"h_{c0}_{ht}", tag="h")
                    for ht in range(KH)
                ]
                for c0 in cts
            ]

            # ---- stage A: gate/up matmuls + silu*mul -> h ----
            for ht4 in range(NH4):
                gts, uts = [], []
                for kt in range(KD):
                    gt = wpool.tile([P, 512], F32R, name=f"g_{ht4}_{kt}", tag="w")
                    nc.sync.dma_start(
                        gt[:, :], gwT[kt * P:(kt + 1) * P, ht4 * 512:(ht4 + 1) * 512]
                    )
                    ut = wpool.tile([P, 512], F32R, name=f"u_{ht4}_{kt}", tag="w")
                    nc.sync.dma_start(
                        ut[:, :], uwT[kt * P:(kt + 1) * P, ht4 * 512:(ht4 + 1) * 512]
                    )
                    gts.append(gt)
                    uts.append(ut)
                for sub in range(4):
                    ht = ht4 * 4 + sub
                    for ci in range(len(cts)):
                        pg = pspool.tile([P, CT], F32, name=f"pg_{ht}_{ci}", tag="ps")
                        pu = pspool.tile([P, CT], F32, name=f"pu_{ht}_{ci}", tag="ps")
                        for kt in range(KD):
                            nc.tensor.matmul(
                                pg[:, :],
                                gts[kt][:, sub * P:(sub + 1) * P],
                                xts[ci][kt][:, :],
                                start=(kt == 0),
                                stop=(kt == KD - 1),
                            )
                        for kt in range(KD):
                            nc.tensor.matmul(
                                pu[:, :],
                                uts[kt][:, sub * P:(sub + 1) * P],
                                xts[ci][kt][:, :],
                                start=(kt == 0),
                                stop=(kt == KD - 1),
                            )
                        tmp = tpool.tile([P, CT], F32, name=f"t_{ht}_{ci}", tag="t")
                        nc.scalar.activation(tmp[:, :], pg[:, :], AF.Silu)
                        nc.vector.tensor_mul(hs[ci][ht][:, :], tmp[:, :], pu[:, :])

            # ---- stage B: down matmuls + prob scale -> y ----
            for dc in range(2):
                pos = {}
                for ci in range(len(cts)):
                    for m in range(CT // P):
                        pos[(ci, m)] = pspool.tile(
                            [P, 512], F32, name=f"po_{dc}_{ci}_{m}", tag="ps"
                        )
                for kh in range(KH):
                    dt_ = dpool.tile([P, 512], F32R, name=f"d_{dc}_{kh}", tag="dw")
                    nc.sync.dma_start(
                        dt_[:, :], dwT[kh * P:(kh + 1) * P, dc * 512:(dc + 1) * 512]
                    )
                    for ci in range(len(cts)):
                        for m in range(CT // P):
                            nc.tensor.matmul(
                                pos[(ci, m)][:, :],
                                hs[ci][kh][:, m * P:(m + 1) * P],
                                dt_[:, :],
                                start=(kh == 0),
                                stop=(kh == KH - 1),
                            )
                for ci, c0 in enumerate(cts):
                    for m in range(CT // P):
                        ot = opool.tile([P, 512], F32, name=f"o_{dc}_{ci}_{m}", tag="o")
                        j = c0 // P + m
                        nc.scalar.mul(ot[:, :], pos[(ci, m)][:, :], p_sb[:, j:j + 1])
                        nc.sync.dma_start(
                            y[c0 + m * P:c0 + (m + 1) * P, dc * 512:(dc + 1) * 512],
                            ot[:, :],
                        )


def build_nc(cap=C, reps_loop=False, max_reps=256):
    """Build the per-core Bass program. With reps_loop, the whole body runs
    inside a For_i whose trip count is read from an int32 input "reps"."""
    nc = bacc.Bacc(None, target_bir_lowering=False)
    with tile.TileContext(nc) as tc:
        xt = nc.dram_tensor("xt", [D, cap], F32R, kind="ExternalInput")
        gwT = nc.dram_tensor("gwT", [D, H], F32R, kind="ExternalInput")
        uwT = nc.dram_tensor("uwT", [D, H], F32R, kind="ExternalInput")
        dwT = nc.dram_tensor("dwT", [H, D], F32R, kind="ExternalInput")
        pv = nc.dram_tensor("pv", [P, cap // P], F32, kind="ExternalInput")
        y = nc.dram_tensor("y", [cap, D], F32, kind="ExternalOutput")
        if reps_loop:
            reps = nc.dram_tensor("reps", [1, 1], mybir.dt.int32, kind="ExternalInput")
            with tc.tile_pool(name="rpool", bufs=1) as rpool:
                r_sb = rpool.tile([1, 1], mybir.dt.int32)
                nc.sync.dma_start(r_sb[:, :], reps[:, :])
                rv = nc.values_load(
                    r_sb[0:1, 0:1],
                    min_val=0,
                    max_val=max_reps,
                    skip_runtime_bounds_check=True,
                )
            with tc.For_i(0, rv, 1):
                emit_expert_ffn(tc, xt, gwT, uwT, dwT, pv, y, cap)
        else:
            emit_expert_ffn(tc, xt, gwT, uwT, dwT, pv, y, cap)
    nc.compile()
    return nc


def route_and_dispatch(x, router_w, cap=C):
    """Host router + top-2 dispatch. Returns per-expert packed inputs and
    combine metadata."""
    logits = x @ router_w.T                      # [T, E]
    t_ar = np.arange(T)
    i1 = np.argmax(logits, axis=1)
    l1 = logits[t_ar, i1]
    lm = logits.copy()
    lm[t_ar, i1] = -np.inf
    i2 = np.argmax(lm, axis=1)
    l2 = lm[t_ar, i2]
    e2 = np.exp(l2 - l1)
    p1 = 1.0 / (1.0 + e2)
    p2 = e2 / (1.0 + e2)

    ee = np.concatenate([i1, i2])                # [2T] expert of each pair
    tt = np.concatenate([t_ar, t_ar])            # [2T] token of each pair
    pp = np.concatenate([p1, p2]).astype(np.float32)
    counts = np.bincount(ee, minlength=E)
    starts = np.zeros(E, np.int64)
    starts[1:] = np.cumsum(counts)[:-1]
    order = np.argsort(ee, kind="stable")
    pos = np.empty(2 * T, np.int64)
    pos[order] = np.arange(2 * T) - starts[ee[order]]
    return ee, tt, pp, pos, counts, starts, order


def kernel(**inputs):
    x = np.ascontiguousarray(
        np.asarray(inputs["hidden_states"], np.float32).reshape(T, D)
    )
    router_w = np.asarray(inputs["router_w"], np.float32)
    gate_w = np.asarray(inputs["gate_w"], np.float32)
    up_w = np.asarray(inputs["up_w"], np.float32)
    down_w = np.asarray(inputs["down_w"], np.float32)

    ee, tt, pp, pos, counts, starts, order = route_and_dispatch(x, router_w)

    in_maps = []
    for e in range(E):
        n_e = min(int(counts[e]), C)
        sel = order[starts[e]:starts[e] + n_e]   # pairs dispatched to core e
        xp = np.zeros((C, D), np.float32)
        xp[:n_e] = x[tt[sel]]
        pvec = np.zeros(C, np.float32)
        pvec[:n_e] = pp[sel]
        in_maps.append(
            {
                "xt": np.ascontiguousarray(xp.T),
                "gwT": np.ascontiguousarray(gate_w[e].T),
                "uwT": np.ascontiguousarray(up_w[e].T),
                "dwT": np.ascontiguousarray(down_w[e].T),
                "pv": np.ascontiguousarray(pvec.reshape(C // P, P).T),
            }
        )

    nc = build_nc()
    res = run_bass_kernel_spmd(nc, in_maps, core_ids=list(range(E)))
    ys = np.stack([res.results[e]["y"] for e in range(E)]).reshape(E * C, D)

    ok = pos < C
    contrib = np.zeros((2 * T, D), np.float32)
    g = ee * C + pos
    contrib[ok] = ys[g[ok]]
    # capacity-overflow fallback (never hit for the seed-0 data): exact fp32
    # host compute for pairs beyond capacity
    if not ok.all():
        for j in np.nonzero(~ok)[0]:
            e = int(ee[j])
            xv = x[tt[j]]
            gate = gate_w[e] @ xv
            up = up_w[e] @ xv
            hv = (gate / (1.0 + np.exp(-gate))) * up
            contrib[j] = (down_w[e] @ hv) * pp[j]
    out = contrib[:T] + contrib[T:]
    return out.reshape(B, S, D).astype(np.float32)


# revision 9
# speedup vs baseline: 1.0471x; 1.0222x over previous
"""MoE layer (top-2 of 8 experts) for 8 Trainium2 NeuronCores.

Strategy: expert-parallel. Host computes the (tiny) router + top-2 dispatch in
numpy; core e runs expert e's FFN over its dispatched tokens (padded to a fixed
capacity C) with fp32r matmuls; host combines the two expert outputs per token.

All device matmuls are [K=128]x[M=128]x[N=512] fp32r (1 cycle/row):
  gate^T/up^T [H, Ct] = gwT/uwT.T @ xt   (contraction over D, 8 k-tiles)
  h = silu(gate) * up                    (SBUF-resident [128, 512] tiles)
  y [Ct, D] = (h.T @ dwT) * p            (contraction over H, 16 k-tiles,
                                          combine-prob scale fused in eviction)

DMA on trn2 costs ~2.8us fixed per instruction, so the host pre-packs weights
and tokens into SBUF-tile order: every load is a single multi-MB DMA with 16KB
contiguous lines. Input loads issue on the sync queue, output stores on gpsimd.
"""

import numpy as np

import concourse.bass as bass
import concourse.mybir as mybir
import concourse.tile as tile
from concourse import bacc
from concourse.bass_utils import run_bass_kernel_spmd

E = 8
TOP_K = 2
B, S, D, H = 4, 2048, 1024, 2048
T = B * S
C = 2560          # per-expert token capacity (seed-0 max count is 2175)
CT = 512          # token tile
P = 128
NCT = C // CT     # 5
KD = D // P       # 8  k-tiles for gate/up
KH = H // P       # 16 k-tiles for down
NH4 = H // 512    # 4  groups of 4 h-blocks
F32 = mybir.dt.float32
F32R = mybir.dt.float32r
AF = mybir.ActivationFunctionType


def emit_expert_ffn(tc, xt, gw, uw, dw, pv, y):
    """Emit one expert's FFN.

    DRAM tensors (all pre-packed on host):
      xt [NCT, 128, KD, 512] f32r - tokens, transposed per ct tile
      gw/uw [NH4, 128, KD, 512] f32r - gate/up weights per 4-h-block group
      dw [2, 2, 128, KH//2, 512] f32r - down weights per (dc, kh-half)
      pv [128, C//128] f32 - combine probs (token-partition layout)
      y  [NCT, 2, 128, 4, 512] f32 out - [ct, dc, p, m, 512]
    """
    nc = tc.nc
    # superblocks of up to 2 token tiles sharing one weight pass
    sbs = [list(range(s, min(s + 2, NCT))) for s in range(0, NCT, 2)]

    with (
        tc.tile_pool(name="xpool", bufs=2) as xpool,
        tc.tile_pool(name="wpool", bufs=3) as wpool,
        tc.tile_pool(name="hpool", bufs=36) as hpool,
        tc.tile_pool(name="dpool", bufs=2) as dpool,
        tc.tile_pool(name="tpool", bufs=3) as tpool,
        tc.tile_pool(name="opool", bufs=2) as opool,
        tc.tile_pool(name="ppool", bufs=1) as ppool,
        tc.tile_pool(name="pspool", bufs=8, space="PSUM") as pspool,
    ):
        p_sb = ppool.tile([P, C // P], F32)
        nc.gpsimd.dma_start(p_sb[:, :], pv[:, :])

        for cts in sbs:
            # ---- token tiles: one 2MB DMA per ct ----
            xts = []
            for ct in cts:
                x_t = xpool.tile([P, KD, CT], F32R, name=f"xt_{ct}", tag="xt")
                nc.gpsimd.dma_start(x_t[:, :, :], xt[ct])
                xts.append(x_t)
            hs = [
                [
                    hpool.tile([P, CT], F32R, name=f"h_{ct}_{ht}", tag="h")
                    for ht in range(KH)
                ]
                for ct in cts
            ]

            # ---- stage A: gate/up matmuls + silu*mul -> h ----
            for ht4 in range(NH4):
                gt = wpool.tile([P, KD, 512], F32R, name=f"g_{ht4}", tag="w")
                nc.sync.dma_start(gt[:, :, :], gw[ht4])
                ut = wpool.tile([P, KD, 512], F32R, name=f"u_{ht4}", tag="w")
                nc.sync.dma_start(ut[:, :, :], uw[ht4])
                for sub in range(4):
                    ht = ht4 * 4 + sub
                    for ci in range(len(cts)):
                        pg = pspool.tile([P, CT], F32, name=f"pg_{ht}_{ci}", tag="ps")
                        pu = pspool.tile([P, CT], F32, name=f"pu_{ht}_{ci}", tag="ps")
                        for kt in range(KD):
                            nc.tensor.matmul(
                                pg[:, :],
                                gt[:, kt, sub * P:(sub + 1) * P],
                                xts[ci][:, kt, :],
                                start=(kt == 0),
                                stop=(kt == KD - 1),
                            )
                        for kt in range(KD):
                            nc.tensor.matmul(
                                pu[:, :],
                                ut[:, kt, sub * P:(sub + 1) * P],
                                xts[ci][:, kt, :],
                                start=(kt == 0),
                                stop=(kt == KD - 1),
                            )
                        tmp = tpool.tile([P, CT], F32, name=f"t_{ht}_{ci}", tag="t")
                        nc.scalar.activation(tmp[:, :], pg[:, :], AF.Silu)
                        nc.vector.tensor_mul(hs[ci][ht][:, :], tmp[:, :], pu[:, :])

            # ---- stage B: down matmuls + prob scale -> y ----
            for dc in range(2):
                pos = {}
                for ci in range(len(cts)):
                    for m in range(CT // P):
                        pos[(ci, m)] = pspool.tile(
                            [P, 512], F32, name=f"po_{dc}_{ci}_{m}", tag="ps"
                        )
                ots = [
                    opool.tile([P, CT // P, 512], F32, name=f"o_{dc}_{ci}", tag="o")
                    for ci in range(len(cts))
                ]
                for hf in range(2):
                    dt_ = dpool.tile([P, KH // 2, 512], F32R, name=f"d_{dc}_{hf}", tag="dw")
                    nc.sync.dma_start(dt_[:, :, :], dw[dc, hf])
                    for kb in range(KH // 2):
                        kh = hf * (KH // 2) + kb
                        for ci in range(len(cts)):
                            for m in range(CT // P):
                                nc.tensor.matmul(
                                    pos[(ci, m)][:, :],
                                    hs[ci][kh][:, m * P:(m + 1) * P],
                                    dt_[:, kb, :],
                                    start=(kh == 0),
                                    stop=(kh == KH - 1),
                                )
                for ci, ct in enumerate(cts):
                    for m in range(CT // P):
                        j = ct * (CT // P) + m
                        nc.scalar.mul(
                            ots[ci][:, m, :], pos[(ci, m)][:, :], p_sb[:, j:j + 1]
                        )
                    nc.gpsimd.dma_start(y[ct, dc], ots[ci][:, :, :])


def build_nc(reps_loop=False, max_reps=512):
    """Build the per-core Bass program. With reps_loop, the whole body runs
    inside a For_i whose trip count is read from an int32 input "reps"."""
    nc = bacc.Bacc(None, target_bir_lowering=False)
    with tile.TileContext(nc) as tc:
        xt = nc.dram_tensor("xt", [NCT, P, KD, CT], F32R, kind="ExternalInput")
        gw = nc.dram_tensor("gw", [NH4, P, KD, 512], F32R, kind="ExternalInput")
        uw = nc.dram_tensor("uw", [NH4, P, KD, 512], F32R, kind="ExternalInput")
        dw = nc.dram_tensor("dw", [2, 2, P, KH // 2, 512], F32R, kind="ExternalInput")
        pv = nc.dram_tensor("pv", [P, C // P], F32, kind="ExternalInput")
        y = nc.dram_tensor("y", [NCT, 2, P, CT // P, 512], F32, kind="ExternalOutput")
        if reps_loop:
            reps = nc.dram_tensor("reps", [1, 1], mybir.dt.int32, kind="ExternalInput")
            with tc.tile_pool(name="rpool", bufs=1) as rpool:
                r_sb = rpool.tile([1, 1], mybir.dt.int32)
                nc.sync.dma_start(r_sb[:, :], reps[:, :])
                rv = nc.values_load(
                    r_sb[0:1, 0:1],
                    min_val=0,
                    max_val=max_reps,
                    skip_runtime_bounds_check=True,
                )
            with tc.For_i(0, rv, 1):
                emit_expert_ffn(tc, xt, gw, uw, dw, pv, y)
        else:
            emit_expert_ffn(tc, xt, gw, uw, dw, pv, y)
    nc.compile()
    return nc


def pack_inputs(x_pad, gate_w_e, up_w_e, down_w_e, p_pad):
    """Pack one expert's inputs into the SBUF-tile-order DRAM layouts."""
    # xt [NCT, 128, KD, 512]: [ct, p, kt, tok] = x_pad[ct*512+tok, kt*128+p]
    xt = np.ascontiguousarray(
        x_pad.reshape(NCT, CT, KD, P).transpose(0, 3, 2, 1)
    )
    # gw/uw [NH4, 128, KD, 512]: [b, p, kt, h] = w[b*512+h, kt*128+p]
    gw = np.ascontiguousarray(
        gate_w_e.reshape(NH4, 512, KD, P).transpose(0, 3, 2, 1)
    )
    uw = np.ascontiguousarray(
        up_w_e.reshape(NH4, 512, KD, P).transpose(0, 3, 2, 1)
    )
    # dw [2, 2, 128, KH//2, 512]: [dc, hf, p, kb, d] = down[dc*512+d, hf*1024+kb*128+p]
    dw = np.ascontiguousarray(
        down_w_e.reshape(2, 512, 2, KH // 2, P).transpose(0, 2, 4, 3, 1)
    )
    pv = np.ascontiguousarray(p_pad.reshape(C // P, P).T)
    return {"xt": xt, "gw": gw, "uw": uw, "dw": dw, "pv": pv}


def unpack_y(y_pack):
    """y_pack [NCT, 2, 128, 4, 512] -> y [C, D]."""
    return np.ascontiguousarray(
        y_pack.transpose(0, 3, 2, 1, 4).reshape(C, D)
    )


def route_and_dispatch(x, router_w):
    """Host router + top-2 dispatch (matches softmax/top_k/renorm of the
    reference exactly)."""
    logits = x @ router_w.T                      # [T, E]
    t_ar = np.arange(T)
    i1 = np.argmax(logits, axis=1)
    l1 = logits[t_ar, i1]
    lm = logits.copy()
    lm[t_ar, i1] = -np.inf
    i2 = np.argmax(lm, axis=1)
    l2 = lm[t_ar, i2]
    e2 = np.exp(l2 - l1)
    p1 = 1.0 / (1.0 + e2)
    p2 = e2 / (1.0 + e2)

    ee = np.concatenate([i1, i2])                # [2T] expert of each pair
    tt = np.concatenate([t_ar, t_ar])            # [2T] token of each pair
    pp = np.concatenate([p1, p2]).astype(np.float32)
    counts = np.bincount(ee, minlength=E)
    starts = np.zeros(E, np.int64)
    starts[1:] = np.cumsum(counts)[:-1]
    order = np.argsort(ee, kind="stable")
    pos = np.empty(2 * T, np.int64)
    pos[order] = np.arange(2 * T) - starts[ee[order]]
    return ee, tt, pp, pos, counts, starts, order


def kernel(**inputs):
    x = np.ascontiguousarray(
        np.asarray(inputs["hidden_states"], np.float32).reshape(T, D)
    )
    router_w = np.asarray(inputs["router_w"], np.float32)
    gate_w = np.asarray(inputs["gate_w"], np.float32)
    up_w = np.asarray(inputs["up_w"], np.float32)
    down_w = np.asarray(inputs["down_w"], np.float32)

    ee, tt, pp, pos, counts, starts, order = route_and_dispatch(x, router_w)

    in_maps = []
    for e in range(E):
        n_e = min(int(counts[e]), C)
        sel = order[starts[e]:starts[e] + n_e]   # pairs dispatched to core e
        xp = np.zeros((C, D), np.float32)
        xp[:n_e] = x[tt[sel]]
        pvec = np.zeros(C, np.float32)
        pvec[:n_e] = pp[sel]
        in_maps.append(pack_inputs(xp, gate_w[e], up_w[e], down_w[e], pvec))

    nc = build_nc()
    res = run_bass_kernel_spmd(nc, in_maps, core_ids=list(range(E)))
    ys = np.stack(
        [unpack_y(res.results[e]["y"]) for e in range(E)]
    ).reshape(E * C, D)

    ok = pos < C
    contrib = np.zeros((2 * T, D), np.float32)
    g = ee * C + pos
    contrib[ok] = ys[g[ok]]
    # capacity-overflow fallback (never hit for the seed-0 data): exact fp32
    # host compute for pairs beyond capacity
    if not ok.all():
        for j in np.nonzero(~ok)[0]:
            e = int(ee[j])
            xv = x[tt[j]]
            gate = gate_w[e] @ xv
            up = up_w[e] @ xv
            hv = (gate / (1.0 + np.exp(-gate))) * up
            contrib[j] = (down_w[e] @ hv) * pp[j]
    out = contrib[:T] + contrib[T:]
    return out.reshape(B, S, D).astype(np.float32)


# revision 13
# speedup vs baseline: 1.1692x; 1.1166x over previous
"""MoE layer (top-2 of 8 experts) for 8 Trainium2 NeuronCores.

Strategy: expert-parallel. Host computes the (tiny) router + top-2 dispatch in
numpy; core e runs expert e's FFN over its dispatched tokens (padded to a fixed
capacity C) with fp32r matmuls; host combines the two expert outputs per token.

All device matmuls are [K=128]x[M=128]x[N=512] fp32r (1 cycle/row):
  gate^T/up^T [H, Ct] = gwT/uwT.T @ xt   (contraction over D, 8 k-tiles)
  h = silu(gate) * up                    (SBUF-resident [128, 512] tiles)
  y [Ct, D] = (h.T @ dwT) * p            (contraction over H, 16 k-tiles,
                                          combine-prob scale fused in eviction)

DMA on trn2 costs ~2.8us fixed per instruction, so the host pre-packs weights
and tokens into SBUF-tile order: every load is a single multi-MB DMA with 16KB
contiguous lines. Input loads issue on the sync queue, output stores on gpsimd.
"""

import numpy as np

import concourse.bass as bass
import concourse.mybir as mybir
import concourse.tile as tile
from concourse import bacc
from concourse.bass_utils import run_bass_kernel_spmd

E = 8
TOP_K = 2
B, S, D, H = 4, 2048, 1024, 2048
T = B * S
C = 2560          # per-expert token capacity (seed-0 max count is 2175)
CT = 512          # token tile
P = 128
NCT = C // CT     # 5
KD = D // P       # 8  k-tiles for gate/up
KH = H // P       # 16 k-tiles for down
NH4 = H // 512    # 4  groups of 4 h-blocks
F32 = mybir.dt.float32
F32R = mybir.dt.float32r
AF = mybir.ActivationFunctionType


def emit_expert_ffn(tc, xt, gw, uw, dw, pv, y):
    """Emit one expert's FFN.

    DRAM tensors (all pre-packed on host):
      xt [NCT, 128, KD, 512] f32r - tokens, transposed per ct tile
      gw/uw [NH4, 128, KD, 512] f32r - gate/up weights per 4-h-block group
      dw [2, 2, 128, KH//2, 512] f32r - down weights per (dc, kh-half)
      pv [128, C//128] f32 - combine probs (token-partition layout)
      y  [NCT, 2, 128, 4, 512] f32 out - [ct, dc, p, m, 512]
    """
    nc = tc.nc
    # superblocks of up to 2 token tiles sharing one weight pass
    sbs = [list(range(s, min(s + 2, NCT))) for s in range(0, NCT, 2)]

    with (
        tc.tile_pool(name="xpool", bufs=2) as xpool,
        tc.tile_pool(name="wpool", bufs=3) as wpool,
        tc.tile_pool(name="hpool", bufs=36) as hpool,
        tc.tile_pool(name="dpool", bufs=3) as dpool,
        tc.tile_pool(name="tpool", bufs=5) as tpool,
        tc.tile_pool(name="opool", bufs=2) as opool,
        tc.tile_pool(name="ppool", bufs=1) as ppool,
        tc.tile_pool(name="pspool", bufs=8, space="PSUM") as pspool,
    ):
        p_sb = ppool.tile([P, C // P], F32)
        nc.gpsimd.dma_start(p_sb[:, :], pv[:, :])

        for cts in sbs:
            # ---- token tiles: one 2MB DMA per ct ----
            xts = []
            for ct in cts:
                x_t = xpool.tile([P, KD, CT], F32R, name=f"xt_{ct}", tag="xt")
                nc.gpsimd.dma_start(x_t[:, :, :], xt[ct])
                xts.append(x_t)
            hs = [
                [
                    hpool.tile([P, CT], F32R, name=f"h_{ct}_{ht}", tag="h")
                    for ht in range(KH)
                ]
                for ct in cts
            ]

            # ---- stage A: gate/up matmuls + silu*mul -> h ----
            for ht4 in range(NH4):
                gt = wpool.tile([P, KD, 512], F32R, name=f"g_{ht4}", tag="w")
                nc.sync.dma_start(gt[:, :, :], gw[ht4])
                ut = wpool.tile([P, KD, 512], F32R, name=f"u_{ht4}", tag="w")
                nc.sync.dma_start(ut[:, :, :], uw[ht4])
                # ct-major, all-gate-then-all-up: gt's last use lands at ~75%
                # of the group so the next group's weight DMA overlaps compute
                for ci in range(len(cts)):
                    tmps = []
                    for sub in range(4):
                        ht = ht4 * 4 + sub
                        pg = pspool.tile([P, CT], F32, name=f"pg_{ht}_{ci}", tag="ps")
                        for kt in range(KD):
                            nc.tensor.matmul(
                                pg[:, :],
                                gt[:, kt, sub * P:(sub + 1) * P],
                                xts[ci][:, kt, :],
                                start=(kt == 0),
                                stop=(kt == KD - 1),
                            )
                        tmp = tpool.tile([P, CT], F32, name=f"t_{ht}_{ci}", tag="t")
                        nc.scalar.activation(tmp[:, :], pg[:, :], AF.Silu)
                        tmps.append(tmp)
                    for sub in range(4):
                        ht = ht4 * 4 + sub
                        pu = pspool.tile([P, CT], F32, name=f"pu_{ht}_{ci}", tag="ps")
                        for kt in range(KD):
                            nc.tensor.matmul(
                                pu[:, :],
                                ut[:, kt, sub * P:(sub + 1) * P],
                                xts[ci][:, kt, :],
                                start=(kt == 0),
                                stop=(kt == KD - 1),
                            )
                        nc.vector.tensor_mul(
                            hs[ci][ht][:, :], tmps[sub][:, :], pu[:, :]
                        )

            # ---- stage B: down matmuls + prob scale -> y ----
            for dc in range(2):
                pos = {}
                for ci in range(len(cts)):
                    for m in range(CT // P):
                        pos[(ci, m)] = pspool.tile(
                            [P, 512], F32, name=f"po_{dc}_{ci}_{m}", tag="ps"
                        )
                ots = [
                    opool.tile([P, CT // P, 512], F32, name=f"o_{dc}_{ci}", tag="o")
                    for ci in range(len(cts))
                ]
                for hf in range(4):
                    dt_ = dpool.tile([P, KH // 4, 512], F32R, name=f"d_{dc}_{hf}", tag="dw")
                    nc.sync.dma_start(
                        dt_[:, :, :], dw[dc, hf // 2][:, (hf % 2) * 4:(hf % 2) * 4 + 4, :]
                    )
                    for kb in range(KH // 4):
                        kh = hf * (KH // 4) + kb
                        for ci in range(len(cts)):
                            for m in range(CT // P):
                                nc.tensor.matmul(
                                    pos[(ci, m)][:, :],
                                    hs[ci][kh][:, m * P:(m + 1) * P],
                                    dt_[:, kb, :],
                                    start=(kh == 0),
                                    stop=(kh == KH - 1),
                                )
                for ci, ct in enumerate(cts):
                    for m in range(CT // P):
                        j = ct * (CT // P) + m
                        nc.scalar.mul(
                            ots[ci][:, m, :], pos[(ci, m)][:, :], p_sb[:, j:j + 1]
                        )
                    nc.gpsimd.dma_start(y[ct, dc], ots[ci][:, :, :])


def build_nc(reps_loop=False, max_reps=512):
    """Build the per-core Bass program. With reps_loop, the whole body runs
    inside a For_i whose trip count is read from an int32 input "reps"."""
    nc = bacc.Bacc(None, target_bir_lowering=False)
    with tile.TileContext(nc) as tc:
        xt = nc.dram_tensor("xt", [NCT, P, KD, CT], F32R, kind="ExternalInput")
        gw = nc.dram_tensor("gw", [NH4, P, KD, 512], F32R, kind="ExternalInput")
        uw = nc.dram_tensor("uw", [NH4, P, KD, 512], F32R, kind="ExternalInput")
        dw = nc.dram_tensor("dw", [2, 2, P, KH // 2, 512], F32R, kind="ExternalInput")
        pv = nc.dram_tensor("pv", [P, C // P], F32, kind="ExternalInput")
        y = nc.dram_tensor("y", [NCT, 2, P, CT // P, 512], F32, kind="ExternalOutput")
        if reps_loop:
            reps = nc.dram_tensor("reps", [1, 1], mybir.dt.int32, kind="ExternalInput")
            with tc.tile_pool(name="rpool", bufs=1) as rpool:
                r_sb = rpool.tile([1, 1], mybir.dt.int32)
                nc.sync.dma_start(r_sb[:, :], reps[:, :])
                rv = nc.values_load(
                    r_sb[0:1, 0:1],
                    min_val=0,
                    max_val=max_reps,
                    skip_runtime_bounds_check=True,
                )
            with tc.For_i(0, rv, 1):
                emit_expert_ffn(tc, xt, gw, uw, dw, pv, y)
        else:
            emit_expert_ffn(tc, xt, gw, uw, dw, pv, y)
    nc.compile()
    return nc


def pack_inputs(x_pad, gate_w_e, up_w_e, down_w_e, p_pad):
    """Pack one expert's inputs into the SBUF-tile-order DRAM layouts."""
    # xt [NCT, 128, KD, 512]: [ct, p, kt, tok] = x_pad[ct*512+tok, kt*128+p]
    xt = np.ascontiguousarray(
        x_pad.reshape(NCT, CT, KD, P).transpose(0, 3, 2, 1)
    )
    # gw/uw [NH4, 128, KD, 512]: [b, p, kt, h] = w[b*512+h, kt*128+p]
    gw = np.ascontiguousarray(
        gate_w_e.reshape(NH4, 512, KD, P).transpose(0, 3, 2, 1)
    )
    uw = np.ascontiguousarray(
        up_w_e.reshape(NH4, 512, KD, P).transpose(0, 3, 2, 1)
    )
    # dw [2, 2, 128, KH//2, 512]: [dc, hf, p, kb, d] = down[dc*512+d, hf*1024+kb*128+p]
    dw = np.ascontiguousarray(
        down_w_e.reshape(2, 512, 2, KH // 2, P).transpose(0, 2, 4, 3, 1)
    )
    pv = np.ascontiguousarray(p_pad.reshape(C // P, P).T)
    return {"xt": xt, "gw": gw, "uw": uw, "dw": dw, "pv": pv}


def unpack_y(y_pack):
    """y_pack [NCT, 2, 128, 4, 512] -> y [C, D]."""
    return np.ascontiguousarray(
        y_pack.transpose(0, 3, 2, 1, 4).reshape(C, D)
    )


def route_and_dispatch(x, router_w):
    """Host router + top-2 dispatch (matches softmax/top_k/renorm of the
    reference exactly)."""
    logits = x @ router_w.T                      # [T, E]
    t_ar = np.arange(T)
    i1 = np.argmax(logits, axis=1)
    l1 = logits[t_ar, i1]
    lm = logits.copy()
    lm[t_ar, i1] = -np.inf
    i2 = np.argmax(lm, axis=1)
    l2 = lm[t_ar, i2]
    e2 = np.exp(l2 - l1)
    p1 = 1.0 / (1.0 + e2)
    p2 = e2 / (1.0 + e2)

    ee = np.concatenate([i1, i2])                # [2T] expert of each pair
    tt = np.concatenate([t_ar, t_ar])            # [2T] token of each pair
    pp = np.concatenate([p1, p2]).astype(np.float32)
    counts = np.bincount(ee, minlength=E)
    starts = np.zeros(E, np.int64)
    starts[1:] = np.cumsum(counts)[:-1]
    order = np.argsort(ee, kind="stable")
    pos = np.empty(2 * T, np.int64)
    pos[order] = np.arange(2 * T) - starts[ee[order]]
    return ee, tt, pp, pos, counts, starts, order


def kernel(**inputs):
    x = np.ascontiguousarray(
        np.asarray(inputs["hidden_states"], np.float32).reshape(T, D)
    )
    router_w = np.asarray(inputs["router_w"], np.float32)
    gate_w = np.asarray(inputs["gate_w"], np.float32)
    up_w = np.asarray(inputs["up_w"], np.float32)
    down_w = np.asarray(inputs["down_w"], np.float32)

    ee, tt, pp, pos, counts, starts, order = route_and_dispatch(x, router_w)

    in_maps = []
    for e in range(E):
        n_e = min(int(counts[e]), C)
        sel = order[starts[e]:starts[e] + n_e]   # pairs dispatched to core e
        xp = np.zeros((C, D), np.float32)
        xp[:n_e] = x[tt[sel]]
        pvec = np.zeros(C, np.float32)
        pvec[:n_e] = pp[sel]
        in_maps.append(pack_inputs(xp, gate_w[e], up_w[e], down_w[e], pvec))

    nc = build_nc()
    res = run_bass_kernel_spmd(nc, in_maps, core_ids=list(range(E)))
    ys = np.stack(
        [unpack_y(res.results[e]["y"]) for e in range(E)]
    ).reshape(E * C, D)

    ok = pos < C
    contrib = np.zeros((2 * T, D), np.float32)
    g = ee * C + pos
    contrib[ok] = ys[g[ok]]
    # capacity-overflow fallback (never hit for the seed-0 data): exact fp32
    # host compute for pairs beyond capacity
    if not ok.all():
        for j in np.nonzero(~ok)[0]:
            e = int(ee[j])
            xv = x[tt[j]]
            gate = gate_w[e] @ xv
            up = up_w[e] @ xv
            hv = (gate / (1.0 + np.exp(-gate))) * up
            contrib[j] = (down_w[e] @ hv) * pp[j]
    out = contrib[:T] + contrib[T:]
    return out.reshape(B, S, D).astype(np.float32)


# revision 15
# speedup vs baseline: 1.4522x; 1.2420x over previous
"""MoE layer (top-2 of 8 experts) for 8 Trainium2 NeuronCores.

Strategy: expert-parallel. Host computes the (tiny) router + top-2 dispatch in
numpy; core e runs expert e's FFN over its dispatched tokens (padded to a fixed
capacity C) with fp32r matmuls; host combines the two expert outputs per token.

All device matmuls are [K=128]x[M=128]x[N=512] fp32r (1 cycle/row):
  gate^T/up^T [H, Ct] = gwT/uwT.T @ xt   (contraction over D, 8 k-tiles)
  h = silu(gate) * up                    (SBUF-resident [128, 512] tiles)
  y [Ct, D] = (h.T @ dwT) * p            (contraction over H, 16 k-tiles,
                                          combine-prob scale fused in eviction)

DMA on trn2 costs ~2.8us fixed per instruction, so the host pre-packs weights
and tokens into SBUF-tile order: every load is a single multi-MB DMA with 16KB
contiguous lines. Input loads issue on the sync queue, output stores on gpsimd.
"""

import numpy as np

import concourse.bass as bass
import concourse.mybir as mybir
import concourse.tile as tile
from concourse import bacc
from concourse.bass_utils import run_bass_kernel_spmd

E = 8
TOP_K = 2
B, S, D, H = 4, 2048, 1024, 2048
T = B * S
C = 2048          # per-expert token capacity; overflow pairs (seed-0: ~137
                  # of 16384, counts max 2175) fall back to exact host compute
CT = 512          # token tile
P = 128
NCT = C // CT     # 5
KD = D // P       # 8  k-tiles for gate/up
KH = H // P       # 16 k-tiles for down
NH4 = H // 512    # 4  groups of 4 h-blocks
F32 = mybir.dt.float32
F32R = mybir.dt.float32r
AF = mybir.ActivationFunctionType


def emit_expert_ffn(tc, xt, gw, uw, dw, pv, y):
    """Emit one expert's FFN.

    DRAM tensors (all pre-packed on host):
      xt [NCT, 128, KD, 512] f32r - tokens, transposed per ct tile
      gw/uw [NH4, 128, KD, 512] f32r - gate/up weights per 4-h-block group
      dw [2, 2, 128, KH//2, 512] f32r - down weights per (dc, kh-half)
      pv [128, C//128] f32 - combine probs (token-partition layout)
      y  [NCT, 2, 128, 4, 512] f32 out - [ct, dc, p, m, 512]
    """
    nc = tc.nc
    # superblocks of up to 2 token tiles sharing one weight pass
    sbs = [list(range(s, min(s + 2, NCT))) for s in range(0, NCT, 2)]

    with (
        tc.tile_pool(name="xpool", bufs=2) as xpool,
        tc.tile_pool(name="wpool", bufs=3) as wpool,
        tc.tile_pool(name="hpool", bufs=36) as hpool,
        tc.tile_pool(name="dpool", bufs=3) as dpool,
        tc.tile_pool(name="tpool", bufs=5) as tpool,
        tc.tile_pool(name="opool", bufs=2) as opool,
        tc.tile_pool(name="ppool", bufs=1) as ppool,
        tc.tile_pool(name="pspool", bufs=8, space="PSUM") as pspool,
    ):
        p_sb = ppool.tile([P, C // P], F32)
        nc.gpsimd.dma_start(p_sb[:, :], pv[:, :])

        for cts in sbs:
            # ---- token tiles: one 2MB DMA per ct ----
            xts = []
            for ct in cts:
                x_t = xpool.tile([P, KD, CT], F32R, name=f"xt_{ct}", tag="xt")
                nc.gpsimd.dma_start(x_t[:, :, :], xt[ct])
                xts.append(x_t)
            hs = [
                [
                    hpool.tile([P, CT], F32R, name=f"h_{ct}_{ht}", tag="h")
                    for ht in range(KH)
                ]
                for ct in cts
            ]

            # ---- stage A: gate/up matmuls + silu*mul -> h ----
            for ht4 in range(NH4):
                gt = wpool.tile([P, KD, 512], F32R, name=f"g_{ht4}", tag="w")
                nc.sync.dma_start(gt[:, :, :], gw[ht4])
                ut = wpool.tile([P, KD, 512], F32R, name=f"u_{ht4}", tag="w")
                nc.sync.dma_start(ut[:, :, :], uw[ht4])
                # ct-major, all-gate-then-all-up: gt's last use lands at ~75%
                # of the group so the next group's weight DMA overlaps compute
                for ci in range(len(cts)):
                    tmps = []
                    for sub in range(4):
                        ht = ht4 * 4 + sub
                        pg = pspool.tile([P, CT], F32, name=f"pg_{ht}_{ci}", tag="ps")
                        for kt in range(KD):
                            nc.tensor.matmul(
                                pg[:, :],
                                gt[:, kt, sub * P:(sub + 1) * P],
                                xts[ci][:, kt, :],
                                start=(kt == 0),
                                stop=(kt == KD - 1),
                            )
                        tmp = tpool.tile([P, CT], F32, name=f"t_{ht}_{ci}", tag="t")
                        nc.scalar.activation(tmp[:, :], pg[:, :], AF.Silu)
                        tmps.append(tmp)
                    for sub in range(4):
                        ht = ht4 * 4 + sub
                        pu = pspool.tile([P, CT], F32, name=f"pu_{ht}_{ci}", tag="ps")
                        for kt in range(KD):
                            nc.tensor.matmul(
                                pu[:, :],
                                ut[:, kt, sub * P:(sub + 1) * P],
                                xts[ci][:, kt, :],
                                start=(kt == 0),
                                stop=(kt == KD - 1),
                            )
                        nc.vector.tensor_mul(
                            hs[ci][ht][:, :], tmps[sub][:, :], pu[:, :]
                        )

            # ---- stage B: down matmuls + prob scale -> y ----
            for dc in range(2):
                pos = {}
                for ci in range(len(cts)):
                    for m in range(CT // P):
                        pos[(ci, m)] = pspool.tile(
                            [P, 512], F32, name=f"po_{dc}_{ci}_{m}", tag="ps"
                        )
                ots = [
                    opool.tile([P, CT // P, 512], F32, name=f"o_{dc}_{ci}", tag="o")
                    for ci in range(len(cts))
                ]
                for hf in range(4):
                    dt_ = dpool.tile([P, KH // 4, 512], F32R, name=f"d_{dc}_{hf}", tag="dw")
                    nc.sync.dma_start(
                        dt_[:, :, :], dw[dc, hf // 2][:, (hf % 2) * 4:(hf % 2) * 4 + 4, :]
                    )
                    for kb in range(KH // 4):
                        kh = hf * (KH // 4) + kb
                        for ci in range(len(cts)):
                            for m in range(CT // P):
                                nc.tensor.matmul(
                                    pos[(ci, m)][:, :],
                                    hs[ci][kh][:, m * P:(m + 1) * P],
                                    dt_[:, kb, :],
                                    start=(kh == 0),
                                    stop=(kh == KH - 1),
                                )
                for ci, ct in enumerate(cts):
                    for m in range(CT // P):
                        j = ct * (CT // P) + m
                        nc.scalar.mul(
                            ots[ci][:, m, :], pos[(ci, m)][:, :], p_sb[:, j:j + 1]
                        )
                    nc.gpsimd.dma_start(y[ct, dc], ots[ci][:, :, :])


def build_nc(reps_loop=False, max_reps=512):
    """Build the per-core Bass program. With reps_loop, the whole body runs
    inside a For_i whose trip count is read from an int32 input "reps"."""
    nc = bacc.Bacc(None, target_bir_lowering=False)
    with tile.TileContext(nc) as tc:
        xt = nc.dram_tensor("xt", [NCT, P, KD, CT], F32R, kind="ExternalInput")
        gw = nc.dram_tensor("gw", [NH4, P, KD, 512], F32R, kind="ExternalInput")
        uw = nc.dram_tensor("uw", [NH4, P, KD, 512], F32R, kind="ExternalInput")
        dw = nc.dram_tensor("dw", [2, 2, P, KH // 2, 512], F32R, kind="ExternalInput")
        pv = nc.dram_tensor("pv", [P, C // P], F32, kind="ExternalInput")
        y = nc.dram_tensor("y", [NCT, 2, P, CT // P, 512], F32, kind="ExternalOutput")
        if reps_loop:
            reps = nc.dram_tensor("reps", [1, 1], mybir.dt.int32, kind="ExternalInput")
            with tc.tile_pool(name="rpool", bufs=1) as rpool:
                r_sb = rpool.tile([1, 1], mybir.dt.int32)
                nc.sync.dma_start(r_sb[:, :], reps[:, :])
                rv = nc.values_load(
                    r_sb[0:1, 0:1],
                    min_val=0,
                    max_val=max_reps,
                    skip_runtime_bounds_check=True,
                )
            with tc.For_i(0, rv, 1):
                emit_expert_ffn(tc, xt, gw, uw, dw, pv, y)
        else:
            emit_expert_ffn(tc, xt, gw, uw, dw, pv, y)
    nc.compile()
    return nc


def pack_inputs(x_pad, gate_w_e, up_w_e, down_w_e, p_pad):
    """Pack one expert's inputs into the SBUF-tile-order DRAM layouts."""
    # xt [NCT, 128, KD, 512]: [ct, p, kt, tok] = x_pad[ct*512+tok, kt*128+p]
    xt = np.ascontiguousarray(
        x_pad.reshape(NCT, CT, KD, P).transpose(0, 3, 2, 1)
    )
    # gw/uw [NH4, 128, KD, 512]: [b, p, kt, h] = w[b*512+h, kt*128+p]
    gw = np.ascontiguousarray(
        gate_w_e.reshape(NH4, 512, KD, P).transpose(0, 3, 2, 1)
    )
    uw = np.ascontiguousarray(
        up_w_e.reshape(NH4, 512, KD, P).transpose(0, 3, 2, 1)
    )
    # dw [2, 2, 128, KH//2, 512]: [dc, hf, p, kb, d] = down[dc*512+d, hf*1024+kb*128+p]
    dw = np.ascontiguousarray(
        down_w_e.reshape(2, 512, 2, KH // 2, P).transpose(0, 2, 4, 3, 1)
    )
    pv = np.ascontiguousarray(p_pad.reshape(C // P, P).T)
    return {"xt": xt, "gw": gw, "uw": uw, "dw": dw, "pv": pv}


def unpack_y(y_pack):
    """y_pack [NCT, 2, 128, 4, 512] -> y [C, D]."""
    return np.ascontiguousarray(
        y_pack.transpose(0, 3, 2, 1, 4).reshape(C, D)
    )


def route_and_dispatch(x, router_w):
    """Host router + top-2 dispatch (matches softmax/top_k/renorm of the
    reference exactly)."""
    logits = x @ router_w.T                      # [T, E]
    t_ar = np.arange(T)
    i1 = np.argmax(logits, axis=1)
    l1 = logits[t_ar, i1]
    lm = logits.copy()
    lm[t_ar, i1] = -np.inf
    i2 = np.argmax(lm, axis=1)
    l2 = lm[t_ar, i2]
    e2 = np.exp(l2 - l1)
    p1 = 1.0 / (1.0 + e2)
    p2 = e2 / (1.0 + e2)

    ee = np.concatenate([i1, i2])                # [2T] expert of each pair
    tt = np.concatenate([t_ar, t_ar])            # [2T] token of each pair
    pp = np.concatenate([p1, p2]).astype(np.float32)
    counts = np.bincount(ee, minlength=E)
    starts = np.zeros(E, np.int64)
    starts[1:] = np.cumsum(counts)[:-1]
    order = np.argsort(ee, kind="stable")
    pos = np.empty(2 * T, np.int64)
    pos[order] = np.arange(2 * T) - starts[ee[order]]
    return ee, tt, pp, pos, counts, starts, order


def kernel(**inputs):
    x = np.ascontiguousarray(
        np.asarray(inputs["hidden_states"], np.float32).reshape(T, D)
    )
    router_w = np.asarray(inputs["router_w"], np.float32)
    gate_w = np.asarray(inputs["gate_w"], np.float32)
    up_w = np.asarray(inputs["up_w"], np.float32)
    down_w = np.asarray(inputs["down_w"], np.float32)

    ee, tt, pp, pos, counts, starts, order = route_and_dispatch(x, router_w)

    in_maps = []
    for e in range(E):
        n_e = min(int(counts[e]), C)
        sel = order[starts[e]:starts[e] + n_e]   # pairs dispatched to core e
        xp = np.zeros((C, D), np.float32)
        xp[:n_e] = x[tt[sel]]
        pvec = np.zeros(C, np.float32)
        pvec[:n_e] = pp[sel]
        in_maps.append(pack_inputs(xp, gate_w[e], up_w[e], down_w[e], pvec))

    nc = build_nc()
    res = run_bass_kernel_spmd(nc, in_maps, core_ids=list(range(E)))
    ys = np.stack(
        [unpack_y(res.results[e]["y"]) for e in range(E)]
    ).reshape(E * C, D)

    ok = pos < C
    contrib = np.zeros((2 * T, D), np.float32)
    g = ee * C + pos
    contrib[ok] = ys[g[ok]]
    # capacity-overflow fallback: exact fp32 host compute for the few pairs
    # beyond capacity (~0.8% of pairs for the seed-0 routing), batched per
    # expert
    if not ok.all():
        bad = np.nonzero(~ok)[0]
        for e in np.unique(ee[bad]):
            js = bad[ee[bad] == e]
            xb = x[tt[js]]
            gb = xb @ gate_w[e].T
            ub = xb @ up_w[e].T
            hb = (gb / (1.0 + np.exp(-gb))) * ub
            contrib[js] = (hb @ down_w[e].T) * pp[js, None]
    out = contrib[:T] + contrib[T:]
    return out.reshape(B, S, D).astype(np.float32)


# revision 25
# speedup vs baseline: 1.4887x; 1.0251x over previous
"""MoE layer (top-2 of 8 experts) for 8 Trainium2 NeuronCores.

Strategy: expert-parallel. Host computes the (tiny) router + top-2 dispatch in
numpy; core e runs expert e's FFN over its dispatched tokens (padded to a fixed
capacity C) with fp32r matmuls; host combines the two expert outputs per token.

All device matmuls are [K=128]x[M=128]x[N=512] fp32r (1 cycle/row):
  gate^T/up^T [H, Ct] = gwT/uwT.T @ xt   (contraction over D, 8 k-tiles)
  h = silu(gate) * up                    (SBUF-resident [128, 512] tiles)
  y [Ct, D] = (h.T @ dwT) * p            (contraction over H, 16 k-tiles,
                                          combine-prob scale fused in eviction)

DMA on trn2 costs ~2.8us fixed per instruction, so the host pre-packs weights
and tokens into SBUF-tile order: every load is a single multi-MB DMA with 16KB
contiguous lines. Input loads issue on the sync queue, output stores on gpsimd.
"""

import numpy as np

import concourse.bass as bass
import concourse.mybir as mybir
import concourse.tile as tile
from concourse import bacc
from concourse.bass_utils import run_bass_kernel_spmd

E = 8
TOP_K = 2
B, S, D, H = 4, 2048, 1024, 2048
T = B * S
C = 2048          # per-expert token capacity; overflow pairs (seed-0: ~137
                  # of 16384, counts max 2175) fall back to exact host compute
CT = 512          # token tile
P = 128
NCT = C // CT     # 5
KD = D // P       # 8  k-tiles for gate/up
KH = H // P       # 16 k-tiles for down
NH4 = H // 512    # 4  groups of 4 h-blocks
F32 = mybir.dt.float32
F32R = mybir.dt.float32r
AF = mybir.ActivationFunctionType


def emit_expert_ffn(tc, xt, gw, uw, dw, pv, y):
    """Emit one expert's FFN.

    DRAM tensors (all pre-packed on host):
      xt [NCT, 128, KD, 512] f32r - tokens, transposed per ct tile
      gw/uw [NH4, 128, KD, 512] f32r - gate/up weights per 4-h-block group
      dw [2, 2, 128, KH//2, 512] f32r - down weights per (dc, kh-half)
      pv [128, C//128] f32 - combine probs (token-partition layout)
      y  [NCT, 2, 128, 4, 512] f32 out - [ct, dc, p, m, 512]
    """
    nc = tc.nc
    # superblocks of up to 2 token tiles sharing one weight pass
    sbs = [list(range(s, min(s + 2, NCT))) for s in range(0, NCT, 2)]

    with (
        tc.tile_pool(name="xpool", bufs=2) as xpool,
        tc.tile_pool(name="wpool", bufs=3) as wpool,
        tc.tile_pool(name="hpool", bufs=36) as hpool,
        tc.tile_pool(name="dpool", bufs=3) as dpool,
        tc.tile_pool(name="tpool", bufs=5) as tpool,
        tc.tile_pool(name="opool", bufs=2) as opool,
        tc.tile_pool(name="ppool", bufs=1) as ppool,
        tc.tile_pool(name="pspool", bufs=8, space="PSUM") as pspool,
    ):
        p_sb = ppool.tile([P, C // P], F32)
        nc.gpsimd.dma_start(p_sb[:, :], pv[:, :])

        for cts in sbs:
            # ---- token tiles: one 2MB DMA per ct ----
            xts = []
            for ct in cts:
                x_t = xpool.tile([P, KD, CT], F32R, name=f"xt_{ct}", tag="xt")
                nc.gpsimd.dma_start(x_t[:, 0:4, :], xt[ct][:, 0:4, :])
                nc.gpsimd.dma_start(x_t[:, 4:8, :], xt[ct][:, 4:8, :])
                xts.append(x_t)
            hs = [
                [
                    hpool.tile([P, CT], F32R, name=f"h_{ct}_{ht}", tag="h")
                    for ht in range(KH)
                ]
                for ct in cts
            ]

            # ---- stage A: gate/up matmuls + silu*mul -> h ----
            first_sb = cts[0] == 0
            for ht4 in range(NH4):
                gt = wpool.tile([P, KD, 512], F32R, name=f"g_{ht4}", tag="w")
                if ht4 == 0 and first_sb:
                    # quarter-granularity on the very first load so the first
                    # matmuls start ~2us earlier out of the cold start
                    for q in range(4):
                        nc.sync.dma_start(
                            gt[:, 2 * q:2 * q + 2, :], gw[ht4][:, 2 * q:2 * q + 2, :]
                        )
                else:
                    nc.sync.dma_start(gt[:, 0:4, :], gw[ht4][:, 0:4, :])
                    nc.sync.dma_start(gt[:, 4:8, :], gw[ht4][:, 4:8, :])
                ut = wpool.tile([P, KD, 512], F32R, name=f"u_{ht4}", tag="w")
                nc.scalar.dma_start(ut[:, 0:4, :], uw[ht4][:, 0:4, :])
                nc.scalar.dma_start(ut[:, 4:8, :], uw[ht4][:, 4:8, :])
                # ct-major, all-gate-then-all-up: gt's last use lands at ~75%
                # of the group so the next group's weight DMA overlaps compute
                for ci in range(len(cts)):
                    tmps = []
                    for sub in range(4):
                        ht = ht4 * 4 + sub
                        pg = pspool.tile([P, CT], F32, name=f"pg_{ht}_{ci}", tag="ps")
                        for kt in range(KD):
                            nc.tensor.matmul(
                                pg[:, :],
                                gt[:, kt, sub * P:(sub + 1) * P],
                                xts[ci][:, kt, :],
                                start=(kt == 0),
                                stop=(kt == KD - 1),
                            )
                        tmp = tpool.tile([P, CT], F32, name=f"t_{ht}_{ci}", tag="t")
                        nc.scalar.activation(tmp[:, :], pg[:, :], AF.Silu)
                        tmps.append(tmp)
                    for sub in range(4):
                        ht = ht4 * 4 + sub
                        pu = pspool.tile([P, CT], F32, name=f"pu_{ht}_{ci}", tag="ps")
                        for kt in range(KD):
                            nc.tensor.matmul(
                                pu[:, :],
                                ut[:, kt, sub * P:(sub + 1) * P],
                                xts[ci][:, kt, :],
                                start=(kt == 0),
                                stop=(kt == KD - 1),
                            )
                        nc.vector.tensor_mul(
                            hs[ci][ht][:, :], tmps[sub][:, :], pu[:, :]
                        )

            # ---- stage B: down matmuls + prob scale -> y ----
            for dc in range(2):
                pos = {}
                for ci in range(len(cts)):
                    for m in range(CT // P):
                        pos[(ci, m)] = pspool.tile(
                            [P, 512], F32, name=f"po_{dc}_{ci}_{m}", tag="ps"
                        )
                ots = [
                    opool.tile([P, CT // P, 512], F32, name=f"o_{dc}_{ci}", tag="o")
                    for ci in range(len(cts))
                ]
                for hf in range(4):
                    dt_ = dpool.tile([P, KH // 4, 512], F32R, name=f"d_{dc}_{hf}", tag="dw")
                    nc.gpsimd.dma_start(
                        dt_[:, :, :], dw[dc, hf // 2][:, (hf % 2) * 4:(hf % 2) * 4 + 4, :]
                    )
                    for kb in range(KH // 4):
                        kh = hf * (KH // 4) + kb
                        for ci in range(len(cts)):
                            for m in range(CT // P):
                                nc.tensor.matmul(
                                    pos[(ci, m)][:, :],
                                    hs[ci][kh][:, m * P:(m + 1) * P],
                                    dt_[:, kb, :],
                                    start=(kh == 0),
                                    stop=(kh == KH - 1),
                                )
                for ci, ct in enumerate(cts):
                    for m in range(CT // P):
                        j = ct * (CT // P) + m
                        nc.scalar.mul(
                            ots[ci][:, m, :], pos[(ci, m)][:, :], p_sb[:, j:j + 1]
                        )
                        # per-m stores start as soon as each eviction lands,
                        # shortening the kernel-tail drain
                        nc.gpsimd.dma_start(y[ct, dc][:, m, :], ots[ci][:, m, :])


def build_nc(reps_loop=False, max_reps=512):
    """Build the per-core Bass program. With reps_loop, the whole body runs
    inside a For_i whose trip count is read from an int32 input "reps"."""
    nc = bacc.Bacc(None, target_bir_lowering=False)
    with tile.TileContext(nc) as tc:
        xt = nc.dram_tensor("xt", [NCT, P, KD, CT], F32R, kind="ExternalInput")
        gw = nc.dram_tensor("gw", [NH4, P, KD, 512], F32R, kind="ExternalInput")
        uw = nc.dram_tensor("uw", [NH4, P, KD, 512], F32R, kind="ExternalInput")
        dw = nc.dram_tensor("dw", [2, 2, P, KH // 2, 512], F32R, kind="ExternalInput")
        pv = nc.dram_tensor("pv", [P, C // P], F32, kind="ExternalInput")
        y = nc.dram_tensor("y", [NCT, 2, P, CT // P, 512], F32, kind="ExternalOutput")
        if reps_loop:
            reps = nc.dram_tensor("reps", [1, 1], mybir.dt.int32, kind="ExternalInput")
            with tc.tile_pool(name="rpool", bufs=1) as rpool:
                r_sb = rpool.tile([1, 1], mybir.dt.int32)
                nc.sync.dma_start(r_sb[:, :], reps[:, :])
                rv = nc.values_load(
                    r_sb[0:1, 0:1],
                    min_val=0,
                    max_val=max_reps,
                    skip_runtime_bounds_check=True,
                )
            with tc.For_i(0, rv, 1):
                emit_expert_ffn(tc, xt, gw, uw, dw, pv, y)
        else:
            emit_expert_ffn(tc, xt, gw, uw, dw, pv, y)
    nc.compile()
    return nc


def pack_inputs(x_pad, gate_w_e, up_w_e, down_w_e, p_pad):
    """Pack one expert's inputs into the SBUF-tile-order DRAM layouts."""
    # xt [NCT, 128, KD, 512]: [ct, p, kt, tok] = x_pad[ct*512+tok, kt*128+p]
    xt = np.ascontiguousarray(
        x_pad.reshape(NCT, CT, KD, P).transpose(0, 3, 2, 1)
    )
    # gw/uw [NH4, 128, KD, 512]: [b, p, kt, h] = w[b*512+h, kt*128+p]
    gw = np.ascontiguousarray(
        gate_w_e.reshape(NH4, 512, KD, P).transpose(0, 3, 2, 1)
    )
    uw = np.ascontiguousarray(
        up_w_e.reshape(NH4, 512, KD, P).transpose(0, 3, 2, 1)
    )
    # dw [2, 2, 128, KH//2, 512]: [dc, hf, p, kb, d] = down[dc*512+d, hf*1024+kb*128+p]
    dw = np.ascontiguousarray(
        down_w_e.reshape(2, 512, 2, KH // 2, P).transpose(0, 2, 4, 3, 1)
    )
    pv = np.ascontiguousarray(p_pad.reshape(C // P, P).T)
    return {"xt": xt, "gw": gw, "uw": uw, "dw": dw, "pv": pv}


def unpack_y(y_pack):
    """y_pack [NCT, 2, 128, 4, 512] -> y [C, D]."""
    return np.ascontiguousarray(
        y_pack.transpose(0, 3, 2, 1, 4).reshape(C, D)
    )


def route_and_dispatch(x, router_w):
    """Host router + top-2 dispatch (matches softmax/top_k/renorm of the
    reference exactly)."""
    logits = x @ router_w.T                      # [T, E]
    t_ar = np.arange(T)
    i1 = np.argmax(logits, axis=1)
    l1 = logits[t_ar, i1]
    lm = logits.copy()
    lm[t_ar, i1] = -np.inf
    i2 = np.argmax(lm, axis=1)
    l2 = lm[t_ar, i2]
    e2 = np.exp(l2 - l1)
    p1 = 1.0 / (1.0 + e2)
    p2 = e2 / (1.0 + e2)

    ee = np.concatenate([i1, i2])                # [2T] expert of each pair
    tt = np.concatenate([t_ar, t_ar])            # [2T] token of each pair
    pp = np.concatenate([p1, p2]).astype(np.float32)
    counts = np.bincount(ee, minlength=E)
    starts = np.zeros(E, np.int64)
    starts[1:] = np.cumsum(counts)[:-1]
    order = np.argsort(ee, kind="stable")
    pos = np.empty(2 * T, np.int64)
    pos[order] = np.arange(2 * T) - starts[ee[order]]
    return ee, tt, pp, pos, counts, starts, order


def kernel(**inputs):
    x = np.ascontiguousarray(
        np.asarray(inputs["hidden_states"], np.float32).reshape(T, D)
    )
    router_w = np.asarray(inputs["router_w"], np.float32)
    gate_w = np.asarray(inputs["gate_w"], np.float32)
    up_w = np.asarray(inputs["up_w"], np.float32)
    down_w = np.asarray(inputs["down_w"], np.float32)

    ee, tt, pp, pos, counts, starts, order = route_and_dispatch(x, router_w)

    in_maps = []
    for e in range(E):
        n_e = min(int(counts[e]), C)
        sel = order[starts[e]:starts[e] + n_e]   # pairs dispatched to core e
        xp = np.zeros((C, D), np.float32)
        xp[:n_e] = x[tt[sel]]
        pvec = np.zeros(C, np.float32)
        pvec[:n_e] = pp[sel]
        in_maps.append(pack_inputs(xp, gate_w[e], up_w[e], down_w[e], pvec))

    nc = build_nc()
    res = run_bass_kernel_spmd(nc, in_maps, core_ids=list(range(E)))
    ys = np.stack(
        [unpack_y(res.results[e]["y"]) for e in range(E)]
    ).reshape(E * C, D)

    ok = pos < C
    contrib = np.zeros((2 * T, D), np.float32)
    g = ee * C + pos
    contrib[ok] = ys[g[ok]]
    # capacity-overflow fallback: exact fp32 host compute for the few pairs
    # beyond capacity (~0.8% of pairs for the seed-0 routing), batched per
    # expert
    if not ok.all():
        bad = np.nonzero(~ok)[0]
        for e in np.unique(ee[bad]):
            js = bad[ee[bad] == e]
            xb = x[tt[js]]
            gb = xb @ gate_w[e].T
            ub = xb @ up_w[e].T
            hb = (gb / (1.0 + np.exp(-gb))) * ub
            contrib[js] = (hb @ down_w[e].T) * pp[js, None]
    out = contrib[:T] + contrib[T:]
    return out.reshape(B, S, D).astype(np.float32)
